# revision 9
# baseline (speedup 1.0000x reference)
"""Quanvolutional layer (nn_ConvGenQuantum) as a Trainium2 Bass kernel.

The reference applies, per 2x2 image patch (p0,p1,p2,p3), a fixed 4-qubit
circuit: RY(p_w) encoders, then a fixed 8-gate random layer with params
theta[0..4], then measures <Z_w>. Conjugating each Z_w through the circuit
(Heisenberg picture) and dropping Pauli strings containing Y (the encoded
state is real, so those have zero expectation) collapses the whole circuit
to a closed form:

    q0 = cos(p0 + theta0); q1 = cos(p1); q2 = cos(p2); q3 = cos(p3 + theta3)
    E0 = cos(theta4) * q0
    E1 = cos(theta1) * q0 * q1
    E2 = E1 * q2
    E3 = E2 * q3

(theta2 -- the RZ -- drops out entirely.) Verified exact vs the reference
(rel err ~2.6e-7, fp32 noise).

The ScalarE Sin table only covers [-pi, pi], and pixels are ~N(0,1) with
|p| up to ~5.2, so cos is evaluated via the half-angle identity
    cos(p + B) = 1 - 2*sin((p + B)/2)^2,
whose Sin argument p/2 + B/2 stays inside [-pi, pi] for every input pixel
(the bias for plane 3 is taken as theta3 - pi, flipping the sign of its
cosine, which the final multiply chain absorbs). Per plane:
    u = Sin(0.5*x + B/2)        (ScalarE)
    w = u*u                     (GpSimd)
    m = 1 - 2w = cos(p + B)     (folded into the DVE multiply chain)

The kernel is pure memory streaming: batch is sharded 4096/8 = 512 images
per NeuronCore (pure data parallel, no collectives); per core it DMAs
512x784 floats in, runs 4 Sin + a short multiply chain, and DMAs 512x784
floats out, writing the four expectations interleaved per patch.
"""

import numpy as np

import concourse.bass as bass
import concourse.bacc as bacc
import concourse.tile as tile
from concourse import mybir
from concourse.bass_utils import run_bass_kernel_spmd

F32 = mybir.dt.float32
N_CORES = 8
B_TOTAL = 4096
ROWS = B_TOTAL // N_CORES       # images per core
PIX = 784                       # 28*28
N_CHUNKS = 2                    # pipeline chunks per core

LAST_RESULT = None              # BassKernelResults of the most recent run


def _build(th0: float, th1: float, th3: float, th4: float,
           n_chunks: int = N_CHUNKS):
    """Build the per-core Bass program for an x shard of [ROWS, 784]."""
    nc = bacc.Bacc(None, target_bir_lowering=False, debug=False)

    s1 = float(np.cos(th1))
    s4 = float(np.cos(th4))
    # Sin biases per pixel plane: cos(p+B) via 1-2*Sin((p+B)/2)^2.
    # Plane 3 uses B = th3 - pi => computes -cos(p3+th3); sign folded below.
    sin_bias = [float(th0 / 2), 0.0, 0.0, float((th3 - np.pi) / 2)]

    # Register activation-bias constants (non-Copy activation float biases
    # are looked up in nc.const_aps; only 0.0/1.0 are pre-registered).
    for i, val in enumerate(dict.fromkeys(sin_bias)):
        if val in (0.0, 1.0):
            continue
        t = nc.alloc_sbuf_tensor(f"const-bias-{i}", [128, 1], F32)
        nc.gpsimd.memset(t.ap(), val)
        nc.const_aps.aps[(F32, val)] = t.ap()
    nc.all_engine_barrier()

    x = nc.declare_dram_parameter("x", [ROWS, PIX], F32, isOutput=False)
    out = nc.declare_dram_parameter("out", [ROWS, PIX], F32, isOutput=True)

    G = ROWS // 128 // n_chunks  # images per partition per chunk
    assert G * 128 * n_chunks == ROWS
    sub = mybir.AluOpType.subtract
    mult = mybir.AluOpType.mult
    SIN = mybir.ActivationFunctionType.Sin
    COPY = mybir.ActivationFunctionType.Copy

    # row = (c*128 + p)*G + g: chunk-major, G consecutive rows per partition
    xv = x.rearrange("(c p g) m -> c p (g m)", p=128, g=G)
    ov = out.rearrange("(c p g) m -> c p (g m)", p=128, g=G)

    with tile.TileContext(nc) as tc:
        with tc.tile_pool(name="io", bufs=2) as io_pool, \
             tc.tile_pool(name="qp", bufs=2) as q_pool:
            for c in range(n_chunks):
                xt = io_pool.tile([128, G * PIX], F32, tag="x")
                nc.sync.dma_start(out=xt[:, :], in_=xv[c])

                # image pixel (2r+b, 2c+d) at free offset g*784+r*56+b*28+c*2+d
                x6 = xt.rearrange("p (g a b c d) -> p g a b c d",
                                  g=G, a=14, b=2, c=14, d=2)

                ws = []
                for i, (bb, dd) in enumerate([(0, 0), (0, 1), (1, 0), (1, 1)]):
                    u = q_pool.tile([128, G * 196], F32, tag=f"u{i}")
                    uv = u.rearrange("p (g a c) -> p g a c", g=G, a=14, c=14)
                    nc.scalar.activation(uv, x6[:, :, :, bb, :, dd], SIN,
                                         bias=sin_bias[i], scale=0.5)
                    w = q_pool.tile([128, G * 196], F32, tag=f"w{i}")
                    nc.gpsimd.tensor_mul(w[:, :], u[:, :], u[:, :])
                    ws.append(w.rearrange("p (g k) -> p g k", g=G))
                w0, w1, w2, w3 = ws

                ot = io_pool.tile([128, G * PIX], F32, tag="o")
                ow = ot.rearrange("p (g k w) -> p g k w", g=G, k=196, w=4)

                # E0 = s4*(1-2w0) = -2*s4*w0 + s4
                nc.scalar.activation(ow[:, :, :, 0], w0, COPY,
                                     bias=s4, scale=-2.0 * s4)
                # rn = 4*s1*(w0-0.5) = -2*s1*m0
                rn = q_pool.tile([128, G * 196], F32, tag="rn")
                rnv = rn.rearrange("p (g k) -> p g k", g=G)
                nc.vector.tensor_scalar(rnv, w0, 0.5, 4.0 * s1,
                                        op0=sub, op1=mult)
                # E1 = (w1-0.5)*rn = s1*m0*m1
                nc.vector.scalar_tensor_tensor(ow[:, :, :, 1], w1, 0.5, rnv,
                                               op0=sub, op1=mult)
                # c1 = -2*E1
                c1 = q_pool.tile([128, G * 196], F32, tag="c1")
                c1v = c1.rearrange("p (g k) -> p g k", g=G)
                nc.vector.tensor_scalar_mul(c1v, ow[:, :, :, 1], -2.0)
                # E2 = (w2-0.5)*c1 = m2*E1
                nc.vector.scalar_tensor_tensor(ow[:, :, :, 2], w2, 0.5, c1v,
                                               op0=sub, op1=mult)
                # c2 = 2*E2
                c2 = q_pool.tile([128, G * 196], F32, tag="c2")
                c2v = c2.rearrange("p (g k) -> p g k", g=G)
                nc.vector.tensor_scalar_mul(c2v, ow[:, :, :, 2], 2.0)
                # E3 = (w3-0.5)*c2 = -m3*E2 = cos(p3+th3)*E2
                nc.vector.scalar_tensor_tensor(ow[:, :, :, 3], w3, 0.5, c2v,
                                               op0=sub, op1=mult)

                nc.sync.dma_start(out=ov[c], in_=ot[:, :])

    if not nc.is_finalized():
        nc.finalize()
    return nc


def kernel(x: np.ndarray, theta: np.ndarray, _trace: bool = False) -> np.ndarray:
    global LAST_RESULT
    th = np.asarray(theta, dtype=np.float64)
    nc = _build(th0=float(th[0]), th1=float(th[1]), th3=float(th[3]),
                th4=float(th[4]))

    xf = np.ascontiguousarray(
        np.asarray(x, dtype=np.float32).reshape(B_TOTAL, PIX))
    in_maps = [{"x": xf[i * ROWS:(i + 1) * ROWS]} for i in range(N_CORES)]
    res = run_bass_kernel_spmd(nc, in_maps, core_ids=list(range(N_CORES)),
                               trace=_trace)
    LAST_RESULT = res
    out = np.concatenate([res.results[i]["out"] for i in range(N_CORES)],
                         axis=0)
    return np.ascontiguousarray(out.astype(np.float32, copy=False))
